# revision 25
# baseline (speedup 1.0000x reference)
"""Trainium2 Bass kernel for the Decoder problem (8-core SPMD).

Math notes:
  - The reference softmaxes over a size-1 axis => alpha == 1.0 exactly, and
    context = a.sum(axis=1). The attention matmuls (W1, W2, We) are dead code.
  - Pipeline per core:
      phase A: context rows for this core's 8 batch rows (PE ones-matmul
               reduction over t), AllGather -> full context
      phase B: gate-sharded LSTM step (128 of 1024 units per core),
               AllGather h_new^T -> full h_new^T
      phase C: vocab-sharded logits (4000 of 32000 cols per core) + exp,
               AllReduce of exp-sums -> normalized softmax chunk
"""
import numpy as np

import concourse.bacc as bacc
import concourse.bass as bass
import concourse.mybir as mybir
import concourse.tile as tile
from concourse import bass_utils

F32 = mybir.dt.float32
AF = mybir.ActivationFunctionType

B, TX, ADIM = 64, 1024, 1024
UNITS, VOCAB, EDIM = 1024, 32000, 512
NCORES = 8
BS = B // NCORES          # 8 batch rows per core (context phase)
USH = UNITS // NCORES     # 128 LSTM unit columns per core (gate shard)
VSH = VOCAB // NCORES     # 4000 vocab columns per core
KZ = ADIM + EDIM + UNITS  # 2560 contraction dim for z = [ctx, xemb, h] @ W
NZT = KZ // 128           # 20 k-tiles for z
NKT = UNITS // 128        # 8 k-tiles for logits
NNT = 8                   # logits n-tiles per core
NT = VSH // NNT           # 500 cols per n-tile

_CACHE = {}


def _build_program():
    nc = bacc.Bacc("TRN2", target_bir_lowering=False, debug=False,
                   num_devices=NCORES)

    a_sh = nc.dram_tensor("a_sh", [BS, TX, ADIM], F32, kind="ExternalInput")
    hT_d = nc.dram_tensor("hT", [UNITS, B], F32, kind="ExternalInput")
    xembT_d = nc.dram_tensor("xembT", [EDIM, B], F32, kind="ExternalInput")
    eye_d = nc.dram_tensor("eye64", [B, B], F32, kind="ExternalInput")
    Wz_d = nc.dram_tensor("Wz", [KZ, 4 * USH], F32, kind="ExternalInput")
    blz_d = nc.dram_tensor("blz", [1, 4 * USH], F32, kind="ExternalInput")
    csl_d = nc.dram_tensor("csl", [B, USH], F32, kind="ExternalInput")
    Wv_d = nc.dram_tensor("Wv", [UNITS, VSH], F32, kind="ExternalInput")
    bv_d = nc.dram_tensor("bv", [1, VSH], F32, kind="ExternalInput")

    ctx_rows_d = nc.dram_tensor("ctx_rows", [BS, ADIM], F32, kind="ExternalOutput")
    c_new_d = nc.dram_tensor("c_new_sl", [B, USH], F32, kind="ExternalOutput")
    h_new_d = nc.dram_tensor("h_new_sl", [B, USH], F32, kind="ExternalOutput")
    y_d = nc.dram_tensor("y_sl", [B, VSH], F32, kind="ExternalOutput")

    rg = [list(range(NCORES))]

    with tile.TileContext(nc) as tc:
        with tc.tile_pool(name="dram", bufs=1, space="DRAM") as dram, \
             tc.tile_pool(name="const", bufs=1) as constp, \
             tc.tile_pool(name="pers", bufs=1) as pers:

            # -------- DRAM bounce buffers for collectives --------
            # ctx is AllGathered in two halves so the first AG overlaps the
            # second half of phase A
            ctx_bounce0 = dram.tile([BS // 2, ADIM], F32)
            ctx_bounce1 = dram.tile([BS // 2, ADIM], F32)
            ctx_full0 = dram.tile([B // 2, ADIM], F32, addr_space="Shared")
            ctx_full1 = dram.tile([B // 2, ADIM], F32, addr_space="Shared")
            hnT_bounce = dram.tile([USH, B], F32)
            hnT_full = dram.tile([UNITS, B], F32, addr_space="Shared")
            sum_bounce = dram.tile([B, 1], F32)
            sum_full = dram.tile([B, 1], F32, addr_space="Shared")

            # -------- constants --------
            ones_k = constp.tile([128, 1], F32)
            nc.gpsimd.memset(ones_k[:], 1.0)
            ones_b = constp.tile([1, B], F32)
            nc.gpsimd.memset(ones_b[:], 1.0)
            eye_sb = constp.tile([B, B], F32)

            # -------- persistent SBUF tiles --------
            xinT = pers.tile([128, NZT, B], F32)      # xin^T k-tiles
            hfullT = pers.tile([128, NKT, B], F32)    # h_new^T k-tiles
            # context rows live on partition 0 (engines can't write at
            # non-32-aligned partition offsets); DMA scatters to DRAM rows
            ctx_sb = pers.tile([1, BS, ADIM], F32)
            csl_sb = pers.tile([B, USH], F32)
            blz_sb = pers.tile([1, 4 * USH], F32)
            bv_sb = pers.tile([1, VSH], F32)
            exp_sb = pers.tile([B, NNT, NT], F32)
            sums = pers.tile([B, NNT], F32)
            total = pers.tile([B, 1], F32)
            recip = pers.tile([B, 1], F32)

            # ================= phase A: context =================
            # DVE tree-reduces t 1024->128 within each a row-block; the PE
            # only does the final 128-partition reduce (ones matmul), since
            # fp32 PE matmuls run at quarter rate (LOW_HIGH).
            # DMA ring discipline: nc.sync (SP HWDGE ring) carries only
            # never-waiting load streams (a, Wz, Wv) plus the final y store;
            # nc.scalar (ACT HWDGE ring) carries everything whose issue waits
            # on a producer — a waiting head would block the whole ring.
            # Small input loads go first on the scalar ring (they don't
            # compete with the sync-ring a stream).
            nc.scalar.dma_start(eye_sb[:], eye_d[:])
            nc.scalar.dma_start(csl_sb[:], csl_d[:])
            nc.scalar.dma_start(blz_sb[:], blz_d[:])
            nc.scalar.dma_start(bv_sb[:], bv_d[:])
            nc.scalar.dma_start(
                xinT[:, NKT:NKT + EDIM // 128, :],
                xembT_d.rearrange("(n p) b -> p n b", p=128))
            nc.scalar.dma_start(
                xinT[:, NKT + EDIM // 128:NZT, :],
                hT_d.rearrange("(n p) b -> p n b", p=128))
            # preload the sigmoid ACT table set up front so the gate
            # activations don't pay the ~2.7us swap later
            dummy = constp.tile([1, 1], F32)
            nc.scalar.activation(dummy[:], ones_k[0:1, 0:1], AF.Sigmoid)

            with tc.tile_pool(name="apool", bufs=2) as apool, \
                 tc.tile_pool(name="ctxps", bufs=4, space="PSUM") as ctxps:
                for b in range(BS):
                    at = apool.tile([128, TX // 128, ADIM], F32, tag="at")
                    # t = p*8 + n so each partition reads 8 contiguous rows
                    nc.sync.dma_start(
                        at[:], a_sh[b].rearrange("(p n) d -> p n d", n=TX // 128))
                    nc.vector.tensor_add(at[:, 0:4, :], at[:, 0:4, :], at[:, 4:8, :])
                    nc.vector.tensor_add(at[:, 0:2, :], at[:, 0:2, :], at[:, 2:4, :])
                    red = apool.tile([128, ADIM], F32, tag="red")
                    nc.vector.tensor_add(red[:], at[:, 0, :], at[:, 1, :])
                    for ds in range(ADIM // 512):
                        cps = ctxps.tile([1, 512], F32, tag="cps")
                        nc.tensor.matmul(cps[:], ones_k[:],
                                         red[:, ds * 512:(ds + 1) * 512])
                        nc.scalar.copy(ctx_sb[:, b, ds * 512:(ds + 1) * 512], cps[:])
                    if b == BS // 2 - 1:
                        # first half done -> AllGather it now, overlapping
                        # the rest of phase A (emitted here so the in-order
                        # ACT ring issues it before the b4..7 copies)
                        nc.scalar.dma_start(ctx_bounce0[:],
                                            ctx_sb[:, 0:BS // 2, :])
                        nc.gpsimd.collective_compute(
                            "AllGather", mybir.AluOpType.bypass,
                            replica_groups=rg,
                            ins=[ctx_bounce0[:]], outs=[ctx_full0[:]])

            nc.scalar.dma_start(ctx_bounce1[:], ctx_sb[:, BS // 2:BS, :])
            nc.gpsimd.collective_compute(
                "AllGather", mybir.AluOpType.bypass, replica_groups=rg,
                ins=[ctx_bounce1[:]], outs=[ctx_full1[:]])
            nc.scalar.dma_start(ctx_rows_d[:], ctx_sb[:])

            with tc.tile_pool(name="trsb", bufs=1) as trsb, \
                 tc.tile_pool(name="tps", bufs=2, space="PSUM") as tps, \
                 tc.tile_pool(name="zps", bufs=1, space="PSUM") as zps, \
                 tc.tile_pool(name="zpsB", bufs=1, space="PSUM") as zpsB, \
                 tc.tile_pool(name="wzp", bufs=4) as wzp, \
                 tc.tile_pool(name="gat", bufs=1) as gat:

                # transposes for the first context half (global b 0..31)
                # run while the second AllGather is still in flight
                ctx_half_sb0 = trsb.tile([B // 2, ADIM], F32)
                nc.scalar.dma_start(ctx_half_sb0[:], ctx_full0[:])
                for kt in range(NKT):
                    tp = tps.tile([128, B // 2], F32, tag="tp")
                    nc.tensor.transpose(
                        tp[:], ctx_half_sb0[:, kt * 128:(kt + 1) * 128],
                        eye_sb[0:B // 2, 0:B // 2])
                    nc.vector.tensor_copy(xinT[:, kt, 0:B // 2], tp[:])

                # ============ phase B: LSTM gates ============
                # k-tiles 8..19 (xemb, h) only need local inputs — they keep
                # the PE busy while the AllGathers complete.
                z_ps = zps.tile([B, 4 * USH], F32)
                for kt in range(NKT, NZT):
                    wz_t = wzp.tile([128, 4 * USH], F32, tag="wz")
                    nc.sync.dma_start(wz_t[:], Wz_d[kt * 128:(kt + 1) * 128, :])
                    nc.tensor.matmul(z_ps[:], xinT[:, kt, :], wz_t[:],
                                     start=(kt == NKT), stop=False)
                nc.tensor.matmul(z_ps[:], ones_b[:], blz_sb[:],
                                 start=False, stop=False)

                # second context half
                ctx_half_sb1 = trsb.tile([B // 2, ADIM], F32)
                nc.scalar.dma_start(ctx_half_sb1[:], ctx_full1[:])
                for kt in range(NKT):
                    tp = tps.tile([128, B // 2], F32, tag="tp")
                    nc.tensor.transpose(
                        tp[:], ctx_half_sb1[:, kt * 128:(kt + 1) * 128],
                        eye_sb[0:B // 2, 0:B // 2])
                    nc.vector.tensor_copy(xinT[:, kt, B // 2:B], tp[:])

                # post-AG k-tiles 0..7 use the same 2x column packing as
                # phase C: even kt accumulate at array cols 0..63 (bank A,
                # joining the early k-tiles), odd kt at cols 64..127 (bank B)
                z_psB = zpsB.tile([128, 4 * USH], F32)
                for p in range(NKT // 2):
                    wzA = wzp.tile([128, 4 * USH], F32, tag="wz", name=f"wzA{p}")
                    nc.sync.dma_start(wzA[:], Wz_d[2 * p * 128:(2 * p + 1) * 128, :])
                    wzB = wzp.tile([128, 4 * USH], F32, tag="wz", name=f"wzB{p}")
                    nc.sync.dma_start(
                        wzB[:], Wz_d[(2 * p + 1) * 128:(2 * p + 2) * 128, :])
                    last = (p == NKT // 2 - 1)
                    nc.tensor.matmul(z_ps[:], xinT[:, 2 * p, :], wzA[:],
                                     start=False, stop=last)
                    nc.tensor.matmul(z_psB[B:2 * B, :], xinT[:, 2 * p + 1, :],
                                     wzB[:], start=(p == 0), stop=last,
                                     tile_position=(0, 64))

                sig_i = gat.tile([B, USH], F32)
                sig_f = gat.tile([B, USH], F32)
                tanh_g = gat.tile([B, USH], F32)
                sig_o = gat.tile([B, USH], F32)
                t1 = gat.tile([B, USH], F32)
                t2 = gat.tile([B, USH], F32)
                c_new = gat.tile([B, USH], F32)
                tanh_c = gat.tile([B, USH], F32)
                h_new = gat.tile([B, USH], F32)
                hnT_sb = gat.tile([USH, B], F32)
                zb_sb = gat.tile([B, 4 * USH], F32)
                z_sum = gat.tile([B, 4 * USH], F32)

                nc.vector.tensor_copy(zb_sb[:], z_psB[B:2 * B, :])
                nc.vector.tensor_add(z_sum[:], z_ps[:], zb_sb[:])
                nc.scalar.activation(sig_i[:], z_sum[:, 0:USH], AF.Sigmoid)
                nc.scalar.activation(sig_f[:], z_sum[:, USH:2 * USH], AF.Sigmoid)
                nc.scalar.activation(sig_o[:], z_sum[:, 3 * USH:4 * USH], AF.Sigmoid)
                nc.scalar.activation(tanh_g[:], z_sum[:, 2 * USH:3 * USH], AF.Tanh)
                nc.vector.tensor_mul(t1[:], sig_f[:], csl_sb[:])
                nc.vector.tensor_mul(t2[:], sig_i[:], tanh_g[:])
                nc.vector.tensor_add(c_new[:], t1[:], t2[:])
                nc.scalar.activation(tanh_c[:], c_new[:], AF.Tanh)
                nc.vector.tensor_mul(h_new[:], sig_o[:], tanh_c[:])
                nc.scalar.dma_start(c_new_d[:], c_new[:])
                nc.scalar.dma_start(h_new_d[:], h_new[:])

                hn_tp = tps.tile([USH, B], F32, tag="tp")
                nc.tensor.transpose(hn_tp[:], h_new[:], eye_sb[:])
                nc.vector.tensor_copy(hnT_sb[:], hn_tp[:])
                nc.scalar.dma_start(hnT_bounce[:], hnT_sb[:])

                # swap in the exp table set while the h AllGather runs
                nc.scalar.activation(dummy[:], ones_k[0:1, 0:1], AF.Exp)

            nc.gpsimd.collective_compute(
                "AllGather", mybir.AluOpType.bypass, replica_groups=rg,
                ins=[hnT_bounce[:]], outs=[hnT_full[:]])
            nc.scalar.dma_start(hfullT[:],
                                hnT_full.rearrange("(n p) b -> p n b", p=128))

            # ============ phase C: logits + softmax ============
            # 2x column packing: even k-tiles accumulate into PSUM partitions
            # 0..63 (PE array cols 0..63), odd k-tiles into partitions 64..127
            # via tile_position=(0,64) — the two streams use different array
            # halves and overlap, halving the fp32 matmul wall time.
            with tc.tile_pool(name="wvp", bufs=8) as wvp, \
                 tc.tile_pool(name="ysb", bufs=2) as ysb, \
                 tc.tile_pool(name="yps", bufs=1, space="PSUM") as yps:
                y_ps = [yps.tile([128, NT], F32, tag=f"y{nt}", name=f"y_ps{nt}")
                        for nt in range(NNT)]
                # bias joins the even-kt (group A) accumulation
                for nt in range(NNT):
                    nc.tensor.matmul(y_ps[nt][0:B, :], ones_b[:],
                                     bv_sb[:, nt * NT:(nt + 1) * NT],
                                     start=True, stop=False)
                # keep-warm chain: tiny dependent matmuls bridge the PE-idle
                # h-AllGather window so phase C starts at K=8/8 clock
                warm = ysb.tile([B // 2, B // 2], F32, tag="warm", name="warm")
                nc.vector.tensor_copy(warm[:], eye_sb[0:B // 2, 0:B // 2])
                for w in range(10):
                    nc.tensor.matmul(y_ps[NNT - 1][B:B + B // 2, 0:B], warm[:],
                                     eye_sb[0:B // 2, 0:B],
                                     tile_position=(0, 64))
                    nc.vector.tensor_copy(warm[:],
                                          y_ps[NNT - 1][B:B + B // 2, 0:B // 2])
                for p in range(NKT // 2):
                    wvA = wvp.tile([128, VSH], F32, tag="wv", name=f"wvA{p}")
                    nc.sync.dma_start(wvA[:], Wv_d[2 * p * 128:(2 * p + 1) * 128, :])
                    wvB = wvp.tile([128, VSH], F32, tag="wv", name=f"wvB{p}")
                    nc.sync.dma_start(wvB[:],
                                      Wv_d[(2 * p + 1) * 128:(2 * p + 2) * 128, :])
                    last = (p == NKT // 2 - 1)
                    for nt in range(NNT):
                        nc.tensor.matmul(
                            y_ps[nt][0:B, :], hfullT[:, 2 * p, :],
                            wvA[:, nt * NT:(nt + 1) * NT],
                            start=False, stop=last)
                        nc.tensor.matmul(
                            y_ps[nt][B:2 * B, :], hfullT[:, 2 * p + 1, :],
                            wvB[:, nt * NT:(nt + 1) * NT],
                            start=(p == 0), stop=last, tile_position=(0, 64))
                for nt in range(NNT):
                    yb = ysb.tile([B, NT], F32, tag="yb", name=f"yb{nt}")
                    nc.vector.tensor_copy(yb[:], y_ps[nt][B:2 * B, :])
                    ysum = ysb.tile([B, NT], F32, tag="ysum", name=f"ysum{nt}")
                    nc.vector.tensor_add(ysum[:], y_ps[nt][0:B, :], yb[:])
                    nc.scalar.activation(exp_sb[:, nt, :], ysum[:], AF.Exp,
                                         accum_out=sums[:, nt:nt + 1])

                nc.vector.tensor_reduce(total[:], sums[:],
                                        mybir.AxisListType.X, mybir.AluOpType.add)
                nc.scalar.dma_start(sum_bounce[:], total[:])
                nc.gpsimd.collective_compute(
                    "AllReduce", mybir.AluOpType.add, replica_groups=rg,
                    ins=[sum_bounce[:]], outs=[sum_full[:]])
                sum_sb = pers.tile([B, 1], F32)
                nc.scalar.dma_start(sum_sb[:], sum_full[:])
                nc.vector.reciprocal(recip[:], sum_sb[:])
                y_view = y_d.rearrange("b (n t) -> b n t", t=NT)
                for h in range(4):
                    sl = slice(h * 2, h * 2 + 2)
                    nc.vector.tensor_scalar_mul(exp_sb[:, sl, :],
                                                exp_sb[:, sl, :], recip[:])
                    nc.sync.dma_start(y_view[:, sl, :], exp_sb[:, sl, :])

    nc.compile()
    return nc


def _get_program():
    if "nc" not in _CACHE:
        _CACHE["nc"] = _build_program()
    return _CACHE["nc"]


def run(inputs, trace=False):
    nc = _get_program()

    X = np.asarray(inputs["X"]).reshape(-1).astype(np.int64)
    a = np.ascontiguousarray(np.asarray(inputs["a"], dtype=np.float32))
    h = np.asarray(inputs["h"], dtype=np.float32)
    c = np.asarray(inputs["c"], dtype=np.float32)
    emb = np.asarray(inputs["emb"], dtype=np.float32)
    Wx = np.asarray(inputs["Wx"], dtype=np.float32)
    Wh = np.asarray(inputs["Wh"], dtype=np.float32)
    bl = np.asarray(inputs["bl"], dtype=np.float32)
    Wv = np.asarray(inputs["Wv"], dtype=np.float32)
    bv = np.asarray(inputs["bv"], dtype=np.float32)

    xemb = emb[X]                                   # [B, EDIM] host gather
    hT = np.ascontiguousarray(h.T)                  # [UNITS, B]
    xembT = np.ascontiguousarray(xemb.T)            # [EDIM, B]
    eye = np.eye(B, dtype=np.float32)
    Wall = np.concatenate([Wx, Wh], axis=0)         # [KZ, 4*UNITS]

    in_maps = []
    row_map = []
    for k in range(NCORES):
        # core k owns global batch rows {4k..4k+3} u {32+4k..32+4k+3} so the
        # two half-batch AllGathers land in global row order
        rows = (list(range(4 * k, 4 * k + 4))
                + list(range(32 + 4 * k, 32 + 4 * k + 4)))
        row_map.append(rows)
        cols = np.concatenate(
            [np.arange(g * UNITS + k * USH, g * UNITS + (k + 1) * USH)
             for g in range(4)])
        in_maps.append({
            "a_sh": np.ascontiguousarray(a[rows]),
            "hT": hT,
            "xembT": xembT,
            "eye64": eye,
            "Wz": np.ascontiguousarray(Wall[:, cols]),
            "blz": np.ascontiguousarray(bl[cols])[None, :],
            "csl": np.ascontiguousarray(c[:, k * USH:(k + 1) * USH]),
            "Wv": np.ascontiguousarray(Wv[:, k * VSH:(k + 1) * VSH]),
            "bv": np.ascontiguousarray(bv[k * VSH:(k + 1) * VSH])[None, :],
        })

    res = bass_utils.run_bass_kernel_spmd(
        nc, in_maps, core_ids=list(range(NCORES)), trace=trace)

    y_pred = np.concatenate([r["y_sl"] for r in res.results], axis=1)
    context = np.empty((B, ADIM), dtype=np.float32)
    for k in range(NCORES):
        context[row_map[k]] = res.results[k]["ctx_rows"]
    context = context[:, None, :]
    h_new = np.concatenate([r["h_new_sl"] for r in res.results], axis=1)
    c_new = np.concatenate([r["c_new_sl"] for r in res.results], axis=1)
    alpha = np.ones((B, TX), dtype=np.float32)
    return (y_pred, context, alpha, h_new, c_new), res


def kernel(**inputs):
    outs, _ = run(inputs, trace=False)
    return outs


# revision 26
# speedup vs baseline: 1.0014x; 1.0014x over previous
"""Trainium2 Bass kernel for the Decoder problem (8-core SPMD).

Math notes:
  - The reference softmaxes over a size-1 axis => alpha == 1.0 exactly, and
    context = a.sum(axis=1). The attention matmuls (W1, W2, We) are dead code.
  - Pipeline per core:
      phase A: context rows for this core's 8 batch rows (PE ones-matmul
               reduction over t), AllGather -> full context
      phase B: gate-sharded LSTM step (128 of 1024 units per core),
               AllGather h_new^T -> full h_new^T
      phase C: vocab-sharded logits (4000 of 32000 cols per core) + exp,
               AllReduce of exp-sums -> normalized softmax chunk
"""
import numpy as np

import concourse.bacc as bacc
import concourse.bass as bass
import concourse.mybir as mybir
import concourse.tile as tile
from concourse import bass_utils

F32 = mybir.dt.float32
AF = mybir.ActivationFunctionType

B, TX, ADIM = 64, 1024, 1024
UNITS, VOCAB, EDIM = 1024, 32000, 512
NCORES = 8
BS = B // NCORES          # 8 batch rows per core (context phase)
USH = UNITS // NCORES     # 128 LSTM unit columns per core (gate shard)
VSH = VOCAB // NCORES     # 4000 vocab columns per core
KZ = ADIM + EDIM + UNITS  # 2560 contraction dim for z = [ctx, xemb, h] @ W
NZT = KZ // 128           # 20 k-tiles for z
NKT = UNITS // 128        # 8 k-tiles for logits
NNT = 8                   # logits n-tiles per core
NT = VSH // NNT           # 500 cols per n-tile

_CACHE = {}


def _build_program():
    nc = bacc.Bacc("TRN2", target_bir_lowering=False, debug=False,
                   num_devices=NCORES)

    a_sh = nc.dram_tensor("a_sh", [BS, TX, ADIM], F32, kind="ExternalInput")
    hT_d = nc.dram_tensor("hT", [UNITS, B], F32, kind="ExternalInput")
    xembT_d = nc.dram_tensor("xembT", [EDIM, B], F32, kind="ExternalInput")
    eye_d = nc.dram_tensor("eye64", [B, B], F32, kind="ExternalInput")
    Wz_d = nc.dram_tensor("Wz", [KZ, 4 * USH], F32, kind="ExternalInput")
    blz_d = nc.dram_tensor("blz", [1, 4 * USH], F32, kind="ExternalInput")
    csl_d = nc.dram_tensor("csl", [B, USH], F32, kind="ExternalInput")
    Wv_d = nc.dram_tensor("Wv", [UNITS, VSH], F32, kind="ExternalInput")
    bv_d = nc.dram_tensor("bv", [1, VSH], F32, kind="ExternalInput")

    ctx_rows_d = nc.dram_tensor("ctx_rows", [BS, ADIM], F32, kind="ExternalOutput")
    c_new_d = nc.dram_tensor("c_new_sl", [B, USH], F32, kind="ExternalOutput")
    h_new_d = nc.dram_tensor("h_new_sl", [B, USH], F32, kind="ExternalOutput")
    y_d = nc.dram_tensor("y_sl", [B, VSH], F32, kind="ExternalOutput")

    rg = [list(range(NCORES))]

    with tile.TileContext(nc) as tc:
        with tc.tile_pool(name="dram", bufs=1, space="DRAM") as dram, \
             tc.tile_pool(name="const", bufs=1) as constp, \
             tc.tile_pool(name="pers", bufs=1) as pers:

            # -------- DRAM bounce buffers for collectives --------
            # ctx is AllGathered in two halves so the first AG overlaps the
            # second half of phase A
            ctx_bounce0 = dram.tile([BS // 2, ADIM], F32)
            ctx_bounce1 = dram.tile([BS // 2, ADIM], F32)
            ctx_full0 = dram.tile([B // 2, ADIM], F32, addr_space="Shared")
            ctx_full1 = dram.tile([B // 2, ADIM], F32, addr_space="Shared")
            hnT_bounce = dram.tile([USH, B], F32)
            hnT_full = dram.tile([UNITS, B], F32, addr_space="Shared")
            sum_bounce = dram.tile([B, 1], F32)
            sum_full = dram.tile([B, 1], F32, addr_space="Shared")

            # -------- constants --------
            ones_k = constp.tile([128, 1], F32)
            nc.gpsimd.memset(ones_k[:], 1.0)
            ones_b = constp.tile([1, B], F32)
            nc.gpsimd.memset(ones_b[:], 1.0)
            eye_sb = constp.tile([B, B], F32)

            # -------- persistent SBUF tiles --------
            xinT = pers.tile([128, NZT, B], F32)      # xin^T k-tiles
            hfullT = pers.tile([128, NKT, B], F32)    # h_new^T k-tiles
            # context rows live on partition 0 (engines can't write at
            # non-32-aligned partition offsets); DMA scatters to DRAM rows
            ctx_sb = pers.tile([1, BS, ADIM], F32)
            csl_sb = pers.tile([B, USH], F32)
            blz_sb = pers.tile([1, 4 * USH], F32)
            bv_sb = pers.tile([1, VSH], F32)
            exp_sb = pers.tile([B, NNT, NT], F32)
            sums = pers.tile([B, NNT], F32)
            total = pers.tile([B, 1], F32)
            recip = pers.tile([B, 1], F32)

            # ================= phase A: context =================
            # DVE tree-reduces t 1024->128 within each a row-block; the PE
            # only does the final 128-partition reduce (ones matmul), since
            # fp32 PE matmuls run at quarter rate (LOW_HIGH).
            # DMA ring discipline: nc.sync (SP HWDGE ring) carries only
            # never-waiting load streams (a, Wz, Wv) plus the final y store;
            # nc.scalar (ACT HWDGE ring) carries everything whose issue waits
            # on a producer — a waiting head would block the whole ring.
            # Small input loads go first on the scalar ring (they don't
            # compete with the sync-ring a stream).
            nc.scalar.dma_start(eye_sb[:], eye_d[:])
            nc.scalar.dma_start(csl_sb[:], csl_d[:])
            nc.scalar.dma_start(blz_sb[:], blz_d[:])
            nc.scalar.dma_start(bv_sb[:], bv_d[:])
            nc.scalar.dma_start(
                xinT[:, NKT:NKT + EDIM // 128, :],
                xembT_d.rearrange("(n p) b -> p n b", p=128))
            nc.scalar.dma_start(
                xinT[:, NKT + EDIM // 128:NZT, :],
                hT_d.rearrange("(n p) b -> p n b", p=128))
            # preload the sigmoid ACT table set up front so the gate
            # activations don't pay the ~2.7us swap later
            dummy = constp.tile([1, 1], F32)
            nc.scalar.activation(dummy[:], ones_k[0:1, 0:1], AF.Sigmoid)

            with tc.tile_pool(name="apool", bufs=2) as apool, \
                 tc.tile_pool(name="ctxps", bufs=4, space="PSUM") as ctxps:
                for b in range(BS):
                    at = apool.tile([128, TX // 128, ADIM], F32, tag="at")
                    # t = p*8 + n so each partition reads 8 contiguous rows
                    nc.sync.dma_start(
                        at[:], a_sh[b].rearrange("(p n) d -> p n d", n=TX // 128))
                    nc.vector.tensor_add(at[:, 0:4, :], at[:, 0:4, :], at[:, 4:8, :])
                    nc.vector.tensor_add(at[:, 0:2, :], at[:, 0:2, :], at[:, 2:4, :])
                    red = apool.tile([128, ADIM], F32, tag="red")
                    nc.vector.tensor_add(red[:], at[:, 0, :], at[:, 1, :])
                    for ds in range(ADIM // 512):
                        cps = ctxps.tile([1, 512], F32, tag="cps")
                        nc.tensor.matmul(cps[:], ones_k[:],
                                         red[:, ds * 512:(ds + 1) * 512])
                        nc.scalar.copy(ctx_sb[:, b, ds * 512:(ds + 1) * 512], cps[:])
                    if b == BS // 2 - 1:
                        # first half done -> AllGather it now, overlapping
                        # the rest of phase A (emitted here so the in-order
                        # ACT ring issues it before the b4..7 copies)
                        nc.scalar.dma_start(ctx_bounce0[:],
                                            ctx_sb[:, 0:BS // 2, :])
                        nc.gpsimd.collective_compute(
                            "AllGather", mybir.AluOpType.bypass,
                            replica_groups=rg,
                            ins=[ctx_bounce0[:]], outs=[ctx_full0[:]])

            nc.scalar.dma_start(ctx_bounce1[:], ctx_sb[:, BS // 2:BS, :])
            nc.gpsimd.collective_compute(
                "AllGather", mybir.AluOpType.bypass, replica_groups=rg,
                ins=[ctx_bounce1[:]], outs=[ctx_full1[:]])
            nc.scalar.dma_start(ctx_rows_d[:], ctx_sb[:])

            with tc.tile_pool(name="trsb", bufs=1) as trsb, \
                 tc.tile_pool(name="tps", bufs=2, space="PSUM") as tps, \
                 tc.tile_pool(name="zps", bufs=1, space="PSUM") as zps, \
                 tc.tile_pool(name="zpsB", bufs=1, space="PSUM") as zpsB, \
                 tc.tile_pool(name="wzp", bufs=4) as wzp, \
                 tc.tile_pool(name="gat", bufs=1) as gat:

                # transposes for the first context half (global b 0..31)
                # run while the second AllGather is still in flight
                ctx_half_sb0 = trsb.tile([B // 2, ADIM], F32)
                nc.scalar.dma_start(ctx_half_sb0[:], ctx_full0[:])
                for kt in range(NKT):
                    tp = tps.tile([128, B // 2], F32, tag="tp")
                    nc.tensor.transpose(
                        tp[:], ctx_half_sb0[:, kt * 128:(kt + 1) * 128],
                        eye_sb[0:B // 2, 0:B // 2])
                    nc.vector.tensor_copy(xinT[:, kt, 0:B // 2], tp[:])

                # ============ phase B: LSTM gates ============
                # k-tiles 8..19 (xemb, h) only need local inputs — they keep
                # the PE busy while the AllGathers complete.
                z_ps = zps.tile([B, 4 * USH], F32)
                for kt in range(NKT, NZT):
                    wz_t = wzp.tile([128, 4 * USH], F32, tag="wz")
                    nc.sync.dma_start(wz_t[:], Wz_d[kt * 128:(kt + 1) * 128, :])
                    nc.tensor.matmul(z_ps[:], xinT[:, kt, :], wz_t[:],
                                     start=(kt == NKT), stop=False)
                nc.tensor.matmul(z_ps[:], ones_b[:], blz_sb[:],
                                 start=False, stop=False)

                # second context half
                ctx_half_sb1 = trsb.tile([B // 2, ADIM], F32)
                nc.scalar.dma_start(ctx_half_sb1[:], ctx_full1[:])
                for kt in range(NKT):
                    tp = tps.tile([128, B // 2], F32, tag="tp")
                    nc.tensor.transpose(
                        tp[:], ctx_half_sb1[:, kt * 128:(kt + 1) * 128],
                        eye_sb[0:B // 2, 0:B // 2])
                    nc.vector.tensor_copy(xinT[:, kt, B // 2:B], tp[:])

                # post-AG k-tiles 0..7 use the same 2x column packing as
                # phase C: even kt accumulate at array cols 0..63 (bank A,
                # joining the early k-tiles), odd kt at cols 64..127 (bank B)
                z_psB = zpsB.tile([128, 4 * USH], F32)
                for p in range(NKT // 2):
                    wzA = wzp.tile([128, 4 * USH], F32, tag="wz", name=f"wzA{p}")
                    nc.sync.dma_start(wzA[:], Wz_d[2 * p * 128:(2 * p + 1) * 128, :])
                    wzB = wzp.tile([128, 4 * USH], F32, tag="wz", name=f"wzB{p}")
                    nc.sync.dma_start(
                        wzB[:], Wz_d[(2 * p + 1) * 128:(2 * p + 2) * 128, :])
                    last = (p == NKT // 2 - 1)
                    nc.tensor.matmul(z_ps[:], xinT[:, 2 * p, :], wzA[:],
                                     start=False, stop=last)
                    nc.tensor.matmul(z_psB[B:2 * B, :], xinT[:, 2 * p + 1, :],
                                     wzB[:], start=(p == 0), stop=last,
                                     tile_position=(0, 64))

                sig_i = gat.tile([B, USH], F32)
                sig_f = gat.tile([B, USH], F32)
                tanh_g = gat.tile([B, USH], F32)
                sig_o = gat.tile([B, USH], F32)
                t1 = gat.tile([B, USH], F32)
                t2 = gat.tile([B, USH], F32)
                c_new = gat.tile([B, USH], F32)
                tanh_c = gat.tile([B, USH], F32)
                h_new = gat.tile([B, USH], F32)
                hnT_sb = gat.tile([USH, B], F32)
                zb_sb = gat.tile([B, 4 * USH], F32)
                z_sum = gat.tile([B, 4 * USH], F32)

                nc.vector.tensor_copy(zb_sb[:], z_psB[B:2 * B, :])
                nc.vector.tensor_add(z_sum[:], z_ps[:], zb_sb[:])
                nc.scalar.activation(sig_i[:], z_sum[:, 0:USH], AF.Sigmoid)
                nc.scalar.activation(sig_f[:], z_sum[:, USH:2 * USH], AF.Sigmoid)
                nc.scalar.activation(sig_o[:], z_sum[:, 3 * USH:4 * USH], AF.Sigmoid)
                nc.scalar.activation(tanh_g[:], z_sum[:, 2 * USH:3 * USH], AF.Tanh)
                nc.vector.tensor_mul(t1[:], sig_f[:], csl_sb[:])
                nc.vector.tensor_mul(t2[:], sig_i[:], tanh_g[:])
                nc.vector.tensor_add(c_new[:], t1[:], t2[:])
                nc.scalar.activation(tanh_c[:], c_new[:], AF.Tanh)
                nc.vector.tensor_mul(h_new[:], sig_o[:], tanh_c[:])
                nc.scalar.dma_start(c_new_d[:], c_new[:])
                nc.scalar.dma_start(h_new_d[:], h_new[:])

                hn_tp = tps.tile([USH, B], F32, tag="tp")
                nc.tensor.transpose(hn_tp[:], h_new[:], eye_sb[:])
                nc.vector.tensor_copy(hnT_sb[:], hn_tp[:])
                nc.scalar.dma_start(hnT_bounce[:], hnT_sb[:])

                # swap in the exp table set while the h AllGather runs
                nc.scalar.activation(dummy[:], ones_k[0:1, 0:1], AF.Exp)

            nc.gpsimd.collective_compute(
                "AllGather", mybir.AluOpType.bypass, replica_groups=rg,
                ins=[hnT_bounce[:]], outs=[hnT_full[:]])
            nc.scalar.dma_start(hfullT[:],
                                hnT_full.rearrange("(n p) b -> p n b", p=128))

            # ============ phase C: logits + softmax ============
            # 2x column packing: even k-tiles accumulate into PSUM partitions
            # 0..63 (PE array cols 0..63), odd k-tiles into partitions 64..127
            # via tile_position=(0,64) — the two streams use different array
            # halves and overlap, halving the fp32 matmul wall time.
            with tc.tile_pool(name="wvp", bufs=8) as wvp, \
                 tc.tile_pool(name="ysb", bufs=2) as ysb, \
                 tc.tile_pool(name="yps", bufs=1, space="PSUM") as yps:
                y_ps = [yps.tile([128, NT], F32, tag=f"y{nt}", name=f"y_ps{nt}")
                        for nt in range(NNT)]
                # bias joins the even-kt (group A) accumulation
                for nt in range(NNT):
                    nc.tensor.matmul(y_ps[nt][0:B, :], ones_b[:],
                                     bv_sb[:, nt * NT:(nt + 1) * NT],
                                     start=True, stop=False)

                for p in range(NKT // 2):
                    wvA = wvp.tile([128, VSH], F32, tag="wv", name=f"wvA{p}")
                    nc.sync.dma_start(wvA[:], Wv_d[2 * p * 128:(2 * p + 1) * 128, :])
                    wvB = wvp.tile([128, VSH], F32, tag="wv", name=f"wvB{p}")
                    nc.sync.dma_start(wvB[:],
                                      Wv_d[(2 * p + 1) * 128:(2 * p + 2) * 128, :])
                    last = (p == NKT // 2 - 1)
                    for nt in range(NNT):
                        nc.tensor.matmul(
                            y_ps[nt][0:B, :], hfullT[:, 2 * p, :],
                            wvA[:, nt * NT:(nt + 1) * NT],
                            start=False, stop=last)
                        nc.tensor.matmul(
                            y_ps[nt][B:2 * B, :], hfullT[:, 2 * p + 1, :],
                            wvB[:, nt * NT:(nt + 1) * NT],
                            start=(p == 0), stop=last, tile_position=(0, 64))
                for nt in range(NNT):
                    yb = ysb.tile([B, NT], F32, tag="yb", name=f"yb{nt}")
                    nc.vector.tensor_copy(yb[:], y_ps[nt][B:2 * B, :])
                    ysum = ysb.tile([B, NT], F32, tag="ysum", name=f"ysum{nt}")
                    nc.vector.tensor_add(ysum[:], y_ps[nt][0:B, :], yb[:])
                    nc.scalar.activation(exp_sb[:, nt, :], ysum[:], AF.Exp,
                                         accum_out=sums[:, nt:nt + 1])

                nc.vector.tensor_reduce(total[:], sums[:],
                                        mybir.AxisListType.X, mybir.AluOpType.add)
                nc.scalar.dma_start(sum_bounce[:], total[:])
                nc.gpsimd.collective_compute(
                    "AllReduce", mybir.AluOpType.add, replica_groups=rg,
                    ins=[sum_bounce[:]], outs=[sum_full[:]])
                sum_sb = pers.tile([B, 1], F32)
                nc.scalar.dma_start(sum_sb[:], sum_full[:])
                nc.vector.reciprocal(recip[:], sum_sb[:])
                y_view = y_d.rearrange("b (n t) -> b n t", t=NT)
                for h in range(4):
                    sl = slice(h * 2, h * 2 + 2)
                    nc.vector.tensor_scalar_mul(exp_sb[:, sl, :],
                                                exp_sb[:, sl, :], recip[:])
                    nc.sync.dma_start(y_view[:, sl, :], exp_sb[:, sl, :])

    nc.compile()
    return nc


def _get_program():
    if "nc" not in _CACHE:
        _CACHE["nc"] = _build_program()
    return _CACHE["nc"]


def run(inputs, trace=False):
    nc = _get_program()

    X = np.asarray(inputs["X"]).reshape(-1).astype(np.int64)
    a = np.ascontiguousarray(np.asarray(inputs["a"], dtype=np.float32))
    h = np.asarray(inputs["h"], dtype=np.float32)
    c = np.asarray(inputs["c"], dtype=np.float32)
    emb = np.asarray(inputs["emb"], dtype=np.float32)
    Wx = np.asarray(inputs["Wx"], dtype=np.float32)
    Wh = np.asarray(inputs["Wh"], dtype=np.float32)
    bl = np.asarray(inputs["bl"], dtype=np.float32)
    Wv = np.asarray(inputs["Wv"], dtype=np.float32)
    bv = np.asarray(inputs["bv"], dtype=np.float32)

    xemb = emb[X]                                   # [B, EDIM] host gather
    hT = np.ascontiguousarray(h.T)                  # [UNITS, B]
    xembT = np.ascontiguousarray(xemb.T)            # [EDIM, B]
    eye = np.eye(B, dtype=np.float32)
    Wall = np.concatenate([Wx, Wh], axis=0)         # [KZ, 4*UNITS]

    in_maps = []
    row_map = []
    for k in range(NCORES):
        # core k owns global batch rows {4k..4k+3} u {32+4k..32+4k+3} so the
        # two half-batch AllGathers land in global row order
        rows = (list(range(4 * k, 4 * k + 4))
                + list(range(32 + 4 * k, 32 + 4 * k + 4)))
        row_map.append(rows)
        cols = np.concatenate(
            [np.arange(g * UNITS + k * USH, g * UNITS + (k + 1) * USH)
             for g in range(4)])
        in_maps.append({
            "a_sh": np.ascontiguousarray(a[rows]),
            "hT": hT,
            "xembT": xembT,
            "eye64": eye,
            "Wz": np.ascontiguousarray(Wall[:, cols]),
            "blz": np.ascontiguousarray(bl[cols])[None, :],
            "csl": np.ascontiguousarray(c[:, k * USH:(k + 1) * USH]),
            "Wv": np.ascontiguousarray(Wv[:, k * VSH:(k + 1) * VSH]),
            "bv": np.ascontiguousarray(bv[k * VSH:(k + 1) * VSH])[None, :],
        })

    res = bass_utils.run_bass_kernel_spmd(
        nc, in_maps, core_ids=list(range(NCORES)), trace=trace)

    y_pred = np.concatenate([r["y_sl"] for r in res.results], axis=1)
    context = np.empty((B, ADIM), dtype=np.float32)
    for k in range(NCORES):
        context[row_map[k]] = res.results[k]["ctx_rows"]
    context = context[:, None, :]
    h_new = np.concatenate([r["h_new_sl"] for r in res.results], axis=1)
    c_new = np.concatenate([r["c_new_sl"] for r in res.results], axis=1)
    alpha = np.ones((B, TX), dtype=np.float32)
    return (y_pred, context, alpha, h_new, c_new), res


def kernel(**inputs):
    outs, _ = run(inputs, trace=False)
    return outs


# revision 27
# speedup vs baseline: 1.1144x; 1.1129x over previous
"""Trainium2 Bass kernel for the Decoder problem (8-core SPMD).

Math notes:
  - The reference softmaxes over a size-1 axis => alpha == 1.0 exactly, and
    context = a.sum(axis=1). The attention matmuls (W1, W2, We) are dead code.
  - Pipeline per core:
      phase A: context rows for this core's 8 batch rows (PE ones-matmul
               reduction over t), AllGather -> full context
      phase B: gate-sharded LSTM step (128 of 1024 units per core),
               AllGather h_new^T -> full h_new^T
      phase C: vocab-sharded logits (4000 of 32000 cols per core) + exp,
               AllReduce of exp-sums -> normalized softmax chunk
"""
import numpy as np

import concourse.bacc as bacc
import concourse.bass as bass
import concourse.mybir as mybir
import concourse.tile as tile
from concourse import bass_utils

F32 = mybir.dt.float32
AF = mybir.ActivationFunctionType

B, TX, ADIM = 64, 1024, 1024
UNITS, VOCAB, EDIM = 1024, 32000, 512
NCORES = 8
BS = B // NCORES          # 8 batch rows per core (context phase)
USH = UNITS // NCORES     # 128 LSTM unit columns per core (gate shard)
VSH = VOCAB // NCORES     # 4000 vocab columns per core
KZ = ADIM + EDIM + UNITS  # 2560 contraction dim for z = [ctx, xemb, h] @ W
NZT = KZ // 128           # 20 k-tiles for z
NKT = UNITS // 128        # 8 k-tiles for logits
NNT = 8                   # logits n-tiles per core
NT = VSH // NNT           # 500 cols per n-tile

_CACHE = {}


def _build_program():
    nc = bacc.Bacc("TRN2", target_bir_lowering=False, debug=False,
                   num_devices=NCORES)

    a_sh = nc.dram_tensor("a_sh", [BS, TX, ADIM], F32, kind="ExternalInput")
    hT_d = nc.dram_tensor("hT", [UNITS, B], F32, kind="ExternalInput")
    xembT_d = nc.dram_tensor("xembT", [EDIM, B], F32, kind="ExternalInput")
    eye_d = nc.dram_tensor("eye64", [B, B], F32, kind="ExternalInput")
    Wz_d = nc.dram_tensor("Wz", [KZ, 4 * USH], F32, kind="ExternalInput")
    blz_d = nc.dram_tensor("blz", [1, 4 * USH], F32, kind="ExternalInput")
    csl_d = nc.dram_tensor("csl", [B, USH], F32, kind="ExternalInput")
    Wv_d = nc.dram_tensor("Wv", [UNITS, VSH], F32, kind="ExternalInput")
    bv_d = nc.dram_tensor("bv", [1, VSH], F32, kind="ExternalInput")

    ctx_rows_d = nc.dram_tensor("ctx_rows", [BS, ADIM], F32, kind="ExternalOutput")
    c_new_d = nc.dram_tensor("c_new_sl", [B, USH], F32, kind="ExternalOutput")
    h_new_d = nc.dram_tensor("h_new_sl", [B, USH], F32, kind="ExternalOutput")
    y_d = nc.dram_tensor("y_sl", [B, VSH], F32, kind="ExternalOutput")

    rg = [list(range(NCORES))]

    with tile.TileContext(nc) as tc:
        with tc.tile_pool(name="dram", bufs=1, space="DRAM") as dram, \
             tc.tile_pool(name="const", bufs=1) as constp, \
             tc.tile_pool(name="pers", bufs=1) as pers:

            # -------- DRAM bounce buffers for collectives --------
            # ctx is AllGathered in two halves so the first AG overlaps the
            # second half of phase A
            ctx_bounce0 = dram.tile([BS // 2, ADIM], F32)
            ctx_bounce1 = dram.tile([BS // 2, ADIM], F32)
            ctx_full0 = dram.tile([B // 2, ADIM], F32, addr_space="Shared")
            ctx_full1 = dram.tile([B // 2, ADIM], F32, addr_space="Shared")
            hnT_bounce = dram.tile([USH, B], F32)
            hnT_full = dram.tile([UNITS, B], F32, addr_space="Shared")
            sum_bounce = dram.tile([B, 1], F32)
            sum_full = dram.tile([B, 1], F32, addr_space="Shared")

            # -------- constants --------
            ones_k = constp.tile([128, 1], F32)
            nc.gpsimd.memset(ones_k[:], 1.0)
            ones_b = constp.tile([1, B], F32)
            nc.gpsimd.memset(ones_b[:], 1.0)
            eye_sb = constp.tile([B, B], F32)

            # -------- persistent SBUF tiles --------
            xinT = pers.tile([128, NZT, B], F32)      # xin^T k-tiles
            hfullT = pers.tile([128, NKT, B], F32)    # h_new^T k-tiles
            # context rows live on partition 0 (engines can't write at
            # non-32-aligned partition offsets); DMA scatters to DRAM rows
            ctx_sb = pers.tile([1, BS, ADIM], F32)
            csl_sb = pers.tile([B, USH], F32)
            blz_sb = pers.tile([1, 4 * USH], F32)
            bv_sb = pers.tile([1, VSH], F32)
            exp_sb = pers.tile([B, NNT, NT], F32)
            sums = pers.tile([B, NNT], F32)
            total = pers.tile([B, 1], F32)
            recip = pers.tile([B, 1], F32)

            # ================= phase A: context =================
            # DVE tree-reduces t 1024->128 within each a row-block; the PE
            # only does the final 128-partition reduce (ones matmul), since
            # fp32 PE matmuls run at quarter rate (LOW_HIGH).
            # DMA ring discipline: nc.sync (SP HWDGE ring) carries only
            # never-waiting load streams (a, Wz, Wv) plus the final y store;
            # nc.scalar (ACT HWDGE ring) carries everything whose issue waits
            # on a producer — a waiting head would block the whole ring.
            # Small input loads go first on the scalar ring (they don't
            # compete with the sync-ring a stream).
            nc.scalar.dma_start(eye_sb[:], eye_d[:])
            nc.scalar.dma_start(csl_sb[:], csl_d[:])
            nc.scalar.dma_start(blz_sb[:], blz_d[:])
            nc.scalar.dma_start(bv_sb[:], bv_d[:])
            nc.scalar.dma_start(
                xinT[:, NKT:NKT + EDIM // 128, :],
                xembT_d.rearrange("(n p) b -> p n b", p=128))
            nc.scalar.dma_start(
                xinT[:, NKT + EDIM // 128:NZT, :],
                hT_d.rearrange("(n p) b -> p n b", p=128))
            # preload the sigmoid ACT table set up front so the gate
            # activations don't pay the ~2.7us swap later
            dummy = constp.tile([1, 1], F32)
            nc.scalar.activation(dummy[:], ones_k[0:1, 0:1], AF.Sigmoid)

            with tc.tile_pool(name="apool", bufs=2) as apool, \
                 tc.tile_pool(name="ctxps", bufs=4, space="PSUM") as ctxps:
                for b in range(BS):
                    at = apool.tile([128, TX // 128, ADIM], F32, tag="at")
                    # t = p*8 + n so each partition reads 8 contiguous rows
                    nc.sync.dma_start(
                        at[:], a_sh[b].rearrange("(p n) d -> p n d", n=TX // 128))
                    nc.vector.tensor_add(at[:, 0:4, :], at[:, 0:4, :], at[:, 4:8, :])
                    nc.vector.tensor_add(at[:, 0:2, :], at[:, 0:2, :], at[:, 2:4, :])
                    red = apool.tile([128, ADIM], F32, tag="red")
                    nc.vector.tensor_add(red[:], at[:, 0, :], at[:, 1, :])
                    for ds in range(ADIM // 512):
                        cps = ctxps.tile([1, 512], F32, tag="cps")
                        nc.tensor.matmul(cps[:], ones_k[:],
                                         red[:, ds * 512:(ds + 1) * 512])
                        nc.scalar.copy(ctx_sb[:, b, ds * 512:(ds + 1) * 512], cps[:])
                    if b == BS // 2 - 1:
                        # first half done -> AllGather it now, overlapping
                        # the rest of phase A (emitted here so the in-order
                        # ACT ring issues it before the b4..7 copies)
                        nc.scalar.dma_start(ctx_bounce0[:],
                                            ctx_sb[:, 0:BS // 2, :])
                        nc.gpsimd.collective_compute(
                            "AllGather", mybir.AluOpType.bypass,
                            replica_groups=rg,
                            ins=[ctx_bounce0[:]], outs=[ctx_full0[:]])

            nc.scalar.dma_start(ctx_bounce1[:], ctx_sb[:, BS // 2:BS, :])
            nc.gpsimd.collective_compute(
                "AllGather", mybir.AluOpType.bypass, replica_groups=rg,
                ins=[ctx_bounce1[:]], outs=[ctx_full1[:]])
            nc.scalar.dma_start(ctx_rows_d[:], ctx_sb[:])

            with tc.tile_pool(name="trsb", bufs=1) as trsb, \
                 tc.tile_pool(name="tps", bufs=2, space="PSUM") as tps, \
                 tc.tile_pool(name="zps", bufs=1, space="PSUM") as zps, \
                 tc.tile_pool(name="zpsB", bufs=1, space="PSUM") as zpsB, \
                 tc.tile_pool(name="wzp", bufs=4) as wzp, \
                 tc.tile_pool(name="gat", bufs=1) as gat:

                # transposes for the first context half (global b 0..31)
                # run while the second AllGather is still in flight
                ctx_half_sb0 = trsb.tile([B // 2, ADIM], F32)
                nc.scalar.dma_start(ctx_half_sb0[:], ctx_full0[:])
                for kt in range(NKT):
                    tp = tps.tile([128, B // 2], F32, tag="tp")
                    nc.tensor.transpose(
                        tp[:], ctx_half_sb0[:, kt * 128:(kt + 1) * 128],
                        eye_sb[0:B // 2, 0:B // 2])
                    nc.vector.tensor_copy(xinT[:, kt, 0:B // 2], tp[:])

                # ============ phase B: LSTM gates ============
                # k-tiles 8..19 (xemb, h) only need local inputs — they keep
                # the PE busy while the AllGathers complete.
                z_ps = zps.tile([B, 4 * USH], F32)
                for kt in range(NKT, NZT):
                    wz_t = wzp.tile([128, 4 * USH], F32, tag="wz")
                    nc.sync.dma_start(wz_t[:], Wz_d[kt * 128:(kt + 1) * 128, :])
                    nc.tensor.matmul(z_ps[:], xinT[:, kt, :], wz_t[:],
                                     start=(kt == NKT), stop=False)
                nc.tensor.matmul(z_ps[:], ones_b[:], blz_sb[:],
                                 start=False, stop=False)

                # second context half
                ctx_half_sb1 = trsb.tile([B // 2, ADIM], F32)
                nc.scalar.dma_start(ctx_half_sb1[:], ctx_full1[:])
                for kt in range(NKT):
                    tp = tps.tile([128, B // 2], F32, tag="tp")
                    nc.tensor.transpose(
                        tp[:], ctx_half_sb1[:, kt * 128:(kt + 1) * 128],
                        eye_sb[0:B // 2, 0:B // 2])
                    nc.vector.tensor_copy(xinT[:, kt, B // 2:B], tp[:])

                # post-AG k-tiles 0..7 use the same 2x column packing as
                # phase C: even kt accumulate at array cols 0..63 (bank A,
                # joining the early k-tiles), odd kt at cols 64..127 (bank B)
                z_psB = zpsB.tile([128, 4 * USH], F32)
                for p in range(NKT // 2):
                    wzA = wzp.tile([128, 4 * USH], F32, tag="wz", name=f"wzA{p}")
                    nc.sync.dma_start(wzA[:], Wz_d[2 * p * 128:(2 * p + 1) * 128, :])
                    wzB = wzp.tile([128, 4 * USH], F32, tag="wz", name=f"wzB{p}")
                    nc.sync.dma_start(
                        wzB[:], Wz_d[(2 * p + 1) * 128:(2 * p + 2) * 128, :])
                    last = (p == NKT // 2 - 1)
                    nc.tensor.matmul(z_ps[:], xinT[:, 2 * p, :], wzA[:],
                                     start=False, stop=last)
                    nc.tensor.matmul(z_psB[B:2 * B, :], xinT[:, 2 * p + 1, :],
                                     wzB[:], start=(p == 0), stop=last,
                                     tile_position=(0, 64))

                sig_i = gat.tile([B, USH], F32)
                sig_f = gat.tile([B, USH], F32)
                tanh_g = gat.tile([B, USH], F32)
                sig_o = gat.tile([B, USH], F32)
                t1 = gat.tile([B, USH], F32)
                t2 = gat.tile([B, USH], F32)
                c_new = gat.tile([B, USH], F32)
                tanh_c = gat.tile([B, USH], F32)
                h_new = gat.tile([B, USH], F32)
                hnT_sb = gat.tile([USH, B], F32)
                zb_sb = gat.tile([B, 4 * USH], F32)
                z_sum = gat.tile([B, 4 * USH], F32)

                nc.vector.tensor_copy(zb_sb[:], z_psB[B:2 * B, :])
                nc.vector.tensor_add(z_sum[:], z_ps[:], zb_sb[:])
                nc.scalar.activation(sig_i[:], z_sum[:, 0:USH], AF.Sigmoid)
                nc.scalar.activation(sig_f[:], z_sum[:, USH:2 * USH], AF.Sigmoid)
                nc.scalar.activation(sig_o[:], z_sum[:, 3 * USH:4 * USH], AF.Sigmoid)
                nc.scalar.activation(tanh_g[:], z_sum[:, 2 * USH:3 * USH], AF.Tanh)
                nc.vector.tensor_mul(t1[:], sig_f[:], csl_sb[:])
                nc.vector.tensor_mul(t2[:], sig_i[:], tanh_g[:])
                nc.vector.tensor_add(c_new[:], t1[:], t2[:])
                nc.scalar.activation(tanh_c[:], c_new[:], AF.Tanh)
                nc.vector.tensor_mul(h_new[:], sig_o[:], tanh_c[:])
                nc.scalar.dma_start(c_new_d[:], c_new[:])
                nc.scalar.dma_start(h_new_d[:], h_new[:])

                hn_tp = tps.tile([USH, B], F32, tag="tp")
                nc.tensor.transpose(hn_tp[:], h_new[:], eye_sb[:])
                nc.vector.tensor_copy(hnT_sb[:], hn_tp[:])
                nc.scalar.dma_start(hnT_bounce[:], hnT_sb[:])

                # swap in the exp table set while the h AllGather runs
                nc.scalar.activation(dummy[:], ones_k[0:1, 0:1], AF.Exp)

            nc.gpsimd.collective_compute(
                "AllGather", mybir.AluOpType.bypass, replica_groups=rg,
                ins=[hnT_bounce[:]], outs=[hnT_full[:]])
            for kt in range(NKT):
                eng = nc.scalar if kt % 2 == 0 else nc.gpsimd
                eng.dma_start(hfullT[:, kt, :],
                              hnT_full[kt * 128:(kt + 1) * 128, :])

            # ============ phase C: logits + softmax ============
            # 2x column packing: even k-tiles accumulate into PSUM partitions
            # 0..63 (PE array cols 0..63), odd k-tiles into partitions 64..127
            # via tile_position=(0,64) — the two streams use different array
            # halves and overlap, halving the fp32 matmul wall time.
            with tc.tile_pool(name="wvp", bufs=8) as wvp, \
                 tc.tile_pool(name="ysb", bufs=2) as ysb, \
                 tc.tile_pool(name="yps", bufs=1, space="PSUM") as yps:
                y_ps = [yps.tile([128, NT], F32, tag=f"y{nt}", name=f"y_ps{nt}")
                        for nt in range(NNT)]
                # bias joins the even-kt (group A) accumulation
                for nt in range(NNT):
                    nc.tensor.matmul(y_ps[nt][0:B, :], ones_b[:],
                                     bv_sb[:, nt * NT:(nt + 1) * NT],
                                     start=True, stop=False)

                for p in range(NKT // 2):
                    wvA = wvp.tile([128, VSH], F32, tag="wv", name=f"wvA{p}")
                    nc.sync.dma_start(wvA[:], Wv_d[2 * p * 128:(2 * p + 1) * 128, :])
                    wvB = wvp.tile([128, VSH], F32, tag="wv", name=f"wvB{p}")
                    nc.sync.dma_start(wvB[:],
                                      Wv_d[(2 * p + 1) * 128:(2 * p + 2) * 128, :])
                    last = (p == NKT // 2 - 1)
                    for nt in range(NNT):
                        nc.tensor.matmul(
                            y_ps[nt][0:B, :], hfullT[:, 2 * p, :],
                            wvA[:, nt * NT:(nt + 1) * NT],
                            start=False, stop=last)
                        nc.tensor.matmul(
                            y_ps[nt][B:2 * B, :], hfullT[:, 2 * p + 1, :],
                            wvB[:, nt * NT:(nt + 1) * NT],
                            start=(p == 0), stop=last, tile_position=(0, 64))
                for nt in range(NNT):
                    yb = ysb.tile([B, NT], F32, tag="yb", name=f"yb{nt}")
                    nc.vector.tensor_copy(yb[:], y_ps[nt][B:2 * B, :])
                    ysum = ysb.tile([B, NT], F32, tag="ysum", name=f"ysum{nt}")
                    nc.vector.tensor_add(ysum[:], y_ps[nt][0:B, :], yb[:])
                    nc.scalar.activation(exp_sb[:, nt, :], ysum[:], AF.Exp,
                                         accum_out=sums[:, nt:nt + 1])

                nc.vector.tensor_reduce(total[:], sums[:],
                                        mybir.AxisListType.X, mybir.AluOpType.add)
                nc.scalar.dma_start(sum_bounce[:], total[:])
                nc.gpsimd.collective_compute(
                    "AllReduce", mybir.AluOpType.add, replica_groups=rg,
                    ins=[sum_bounce[:]], outs=[sum_full[:]])
                sum_sb = pers.tile([B, 1], F32)
                nc.scalar.dma_start(sum_sb[:], sum_full[:])
                nc.vector.reciprocal(recip[:], sum_sb[:])
                y_view = y_d.rearrange("b (n t) -> b n t", t=NT)
                for h in range(4):
                    sl = slice(h * 2, h * 2 + 2)
                    nc.vector.tensor_scalar_mul(exp_sb[:, sl, :],
                                                exp_sb[:, sl, :], recip[:])
                    nc.sync.dma_start(y_view[:, sl, :], exp_sb[:, sl, :])

    nc.compile()
    return nc


def _get_program():
    if "nc" not in _CACHE:
        _CACHE["nc"] = _build_program()
    return _CACHE["nc"]


def run(inputs, trace=False):
    nc = _get_program()

    X = np.asarray(inputs["X"]).reshape(-1).astype(np.int64)
    a = np.ascontiguousarray(np.asarray(inputs["a"], dtype=np.float32))
    h = np.asarray(inputs["h"], dtype=np.float32)
    c = np.asarray(inputs["c"], dtype=np.float32)
    emb = np.asarray(inputs["emb"], dtype=np.float32)
    Wx = np.asarray(inputs["Wx"], dtype=np.float32)
    Wh = np.asarray(inputs["Wh"], dtype=np.float32)
    bl = np.asarray(inputs["bl"], dtype=np.float32)
    Wv = np.asarray(inputs["Wv"], dtype=np.float32)
    bv = np.asarray(inputs["bv"], dtype=np.float32)

    xemb = emb[X]                                   # [B, EDIM] host gather
    hT = np.ascontiguousarray(h.T)                  # [UNITS, B]
    xembT = np.ascontiguousarray(xemb.T)            # [EDIM, B]
    eye = np.eye(B, dtype=np.float32)
    Wall = np.concatenate([Wx, Wh], axis=0)         # [KZ, 4*UNITS]

    in_maps = []
    row_map = []
    for k in range(NCORES):
        # core k owns global batch rows {4k..4k+3} u {32+4k..32+4k+3} so the
        # two half-batch AllGathers land in global row order
        rows = (list(range(4 * k, 4 * k + 4))
                + list(range(32 + 4 * k, 32 + 4 * k + 4)))
        row_map.append(rows)
        cols = np.concatenate(
            [np.arange(g * UNITS + k * USH, g * UNITS + (k + 1) * USH)
             for g in range(4)])
        in_maps.append({
            "a_sh": np.ascontiguousarray(a[rows]),
            "hT": hT,
            "xembT": xembT,
            "eye64": eye,
            "Wz": np.ascontiguousarray(Wall[:, cols]),
            "blz": np.ascontiguousarray(bl[cols])[None, :],
            "csl": np.ascontiguousarray(c[:, k * USH:(k + 1) * USH]),
            "Wv": np.ascontiguousarray(Wv[:, k * VSH:(k + 1) * VSH]),
            "bv": np.ascontiguousarray(bv[k * VSH:(k + 1) * VSH])[None, :],
        })

    res = bass_utils.run_bass_kernel_spmd(
        nc, in_maps, core_ids=list(range(NCORES)), trace=trace)

    y_pred = np.concatenate([r["y_sl"] for r in res.results], axis=1)
    context = np.empty((B, ADIM), dtype=np.float32)
    for k in range(NCORES):
        context[row_map[k]] = res.results[k]["ctx_rows"]
    context = context[:, None, :]
    h_new = np.concatenate([r["h_new_sl"] for r in res.results], axis=1)
    c_new = np.concatenate([r["c_new_sl"] for r in res.results], axis=1)
    alpha = np.ones((B, TX), dtype=np.float32)
    return (y_pred, context, alpha, h_new, c_new), res


def kernel(**inputs):
    outs, _ = run(inputs, trace=False)
    return outs
